# revision 13
# baseline (speedup 1.0000x reference)
"""Causal dot-product attention (B=4, S=2048, D=1024) on 8 TRN2 NeuronCores.

v2: host-side rank-1 f32r-rounding correction, balanced causal tiling,
per-partition-contiguous input layout (128-descriptor DMAs), host
pre-rounding to f32r (no cast-DMAs -> any engine can issue loads),
evacuations split across Scalar/Vector engines.

Sharding: batch x query-tile-class. Core c handles batch c//2; the 16
query row-tiles (128 rows each) are split so slot s of class 0 gets tile
15-2s (extent 16-2s chunks) and class 1 gets tile 14-2s (extent 15-2s,
padded to 16-2s) -> one SPMD program, near-balanced work.

Numerics: projections and QK^T run in f32r (11-bit-mantissa RNE, full PE
speed); inputs/weights are pre-rounded to the identical RNE-11 grid on
the host. The dominant rounding error of the K/Q projections is the
rank-1 term  rowsum(x - rne11(x)) (x) colmean(rne11(W)); both factors
are computed on the host and shipped as small inputs DK/DQ/CK/CQ. The
evacuation fuses  out = d*c + psum  into one scalar_tensor_tensor.
1/sqrt(D) is applied inside the exp activation; the causal mask comes in
as a fused (mask*2^24 + logits) op before max-subtraction.
"""
import numpy as np
import concourse.bass as bass
import concourse.mybir as mybir
from concourse import bacc
from concourse.tile import TileContext
from concourse.bass_utils import run_bass_kernel_spmd
from concourse.masks import make_identity

f32 = mybir.dt.float32
f32r = mybir.dt.float32r
bf16 = mybir.dt.bfloat16
u8 = mybir.dt.uint8
AF = mybir.ActivationFunctionType
ALU = mybir.AluOpType

B, S, D = 4, 2048, 1024
SH = 1024                  # query rows per core
NSLOT = 8
EXT = [16, 14, 12, 10, 8, 6, 4, 2]        # key extent per slot, 128-chunks
NB512 = [e // 4 for e in EXT]             # full 512 blocks
NB256 = [(e % 4) // 2 for e in EXT]       # trailing 256 block (0 or 1)
TILES = [[15, 13, 11, 9, 7, 5, 3, 1], [14, 12, 10, 8, 6, 4, 2, 0]]
MOFF = float(2 ** 24)
SCALE = 1.0 / 32.0


def rne11(x):
    """Bit-exact f32r rounding: RNE to 11 mantissa bits."""
    b = np.asarray(x, dtype=np.float32).view(np.uint32).astype(np.uint64)
    half = np.uint64(1 << 11)
    lsb = (b >> np.uint64(12)) & np.uint64(1)
    b2 = ((b + half - np.uint64(1) + lsb) >> np.uint64(12)) << np.uint64(12)
    return b2.astype(np.uint32).view(np.float32)


def relay(xT, nsb):
    """[D, N] -> [nsb, 128, 8, N//nsb] per-partition-contiguous, rne11'd."""
    Dn, N = xT.shape
    w = N // nsb
    out = rne11(xT).reshape(8, 128, nsb, w).transpose(2, 1, 0, 3)
    return np.ascontiguousarray(out)


def relay_w(W):
    """[D, D] -> [2, 4, 128, 8, 128] chunked per dout-group, rne11'd."""
    out = rne11(W).reshape(8, 128, 2, 4, 128).transpose(2, 3, 1, 0, 4)
    return np.ascontiguousarray(out)


def build():
    nc = bacc.Bacc()
    qT = nc.dram_tensor("qT", [2, 128, 8, 512], f32r, kind="ExternalInput")
    kT = nc.dram_tensor("kT", [4, 128, 8, 512], f32r, kind="ExternalInput")
    vT = nc.dram_tensor("vT", [4, 128, 8, 512], f32r, kind="ExternalInput")
    Wq = nc.dram_tensor("Wq", [2, 4, 128, 8, 128], f32r, kind="ExternalInput")
    Wk = nc.dram_tensor("Wk", [2, 4, 128, 8, 128], f32r, kind="ExternalInput")
    Wv = nc.dram_tensor("Wv", [2, 4, 128, 8, 128], f32r, kind="ExternalInput")
    Mu = nc.dram_tensor("Mu", [SH, S], u8, kind="ExternalInput")
    DK = nc.dram_tensor("DK", [128, S], f32, kind="ExternalInput")
    DQ = nc.dram_tensor("DQ", [128, SH], f32, kind="ExternalInput")
    CK = nc.dram_tensor("CK", [128, 8], f32, kind="ExternalInput")
    CQ = nc.dram_tensor("CQ", [128, 8], f32, kind="ExternalInput")
    O = nc.dram_tensor("O", [SH, D], f32, kind="ExternalOutput")

    def load_whalf(pool, W5, half, tag, eng=None):
        w = pool.tile([128, 4, 8, 128], f32r, tag=tag)
        for d4 in range(4):
            (eng or nc.gpsimd).dma_start(out=w[:, d4], in_=W5[half, d4])
        return w

    with TileContext(nc) as tc:
        with tc.tile_pool(name="pers", bufs=1) as pers:
            k1T = pers.tile([128, 8, S], f32r, tag="k1T")      # 64 KB/part
            v1 = pers.tile([128, 16, D], bf16, tag="v1")       # 32 KB/part

            inp = tc.alloc_tile_pool(name="inp", bufs=2, side="left")
            corrK = tc.alloc_tile_pool(name="corr", bufs=1, side="left")
            wk0_p = tc.alloc_tile_pool(name="wk0", bufs=1, side="left")
            wk1_p = tc.alloc_tile_pool(name="wk1", bufs=1, side="left")
            dkp = tc.alloc_tile_pool(name="dkp", bufs=1, side="left")
            wv0_p = tc.alloc_tile_pool(name="wv0", bufs=1, side="right")

            # qkps open from the start: the first attention QK matmuls need
            # no pool-boundary barrier at the Q->attention transition
            qkps = tc.alloc_tile_pool(name="qkps", bufs=2, space="PSUM")
            pps = tc.alloc_tile_pool(name="pps", bufs=6, space="PSUM")

            def load_in(X4, sb, eng=None):
                """One [128, 8, 512] f32r input chunk, single 128-desc DMA."""
                it = inp.tile([128, 8, 512], f32r, tag="inT")
                (eng or nc.sync).dma_start(out=it, in_=X4[sb])
                return it

            # host-computed rank-1 correction factors
            dkb = dkp.tile([128, S], f32, tag="dkb")       # 8 KB/part
            dqb = corrK.tile([128, SH], f32, tag="dqb")     # 4 KB/part
            ck = corrK.tile([128, 8], f32, tag="ck")
            cq = corrK.tile([128, 8], f32, tag="cq")
            # =============== phase K: k1T = Wk^T kT (+ fused correction) ===============
            # startup-gating set spread over the three DMA-capable queues;
            # correction factors queue AFTER it (needed only at first evac)
            wk0 = wk0_p.tile([128, 4, 8, 128], f32r, tag="wk0")
            nc.gpsimd.dma_start(out=wk0[:, 0], in_=Wk[0, 0])
            it0 = inp.tile([128, 8, 512], f32r, tag="inT")
            nc.sync.dma_start(out=it0[:, 0:4, :], in_=kT[0, :, 0:4])
            nc.scalar.dma_start(out=it0[:, 4:8, :], in_=kT[0, :, 4:8])
            nc.gpsimd.dma_start(out=wk0[:, 1], in_=Wk[0, 1])
            nc.sync.dma_start(out=wk0[:, 2], in_=Wk[0, 2])
            nc.scalar.dma_start(out=wk0[:, 3], in_=Wk[0, 3])
            nc.scalar.dma_start(out=ck[:], in_=CK[:, :])
            nc.scalar.dma_start(out=dkb[:, 0:512], in_=DK[:, 0:512])
            wk = [wk0, None]
            wv = [None, None]
            its_k = [it0, None, None, None]

            def first_evac():
                """Everything queued after the gates waits for the first evac,
                so startup-gating transfers (wk0+it0) get the full bandwidth."""
                for eng, tg in ((nc.sync, "gate_s"), (nc.gpsimd, "gate_g"),
                                (nc.scalar, "gate_a")):
                    g = corrK.tile([1, 16], f32r, tag=tg)
                    eng.dma_start(out=g[:], in_=k1T[0:1, 0, 0:16])
                # gated prefetch: inputs on sync, weights+mask on gpsimd
                nc.scalar.dma_start(out=cq[:], in_=CQ[:, :])
                nc.scalar.dma_start(out=dkb[:, 512:], in_=DK[:, 512:])
                nc.scalar.dma_start(out=dqb[:], in_=DQ[:, :])
                wk[1] = load_whalf(wk1_p, Wk, 1, "wk1")
                for sb2 in range(1, 4):
                    its_k[sb2] = load_in(kT, sb2)

            for sb in range(4):
                it = its_k[sb]
                for dout in range(8):
                    ps = pps.tile([128, 512], f32, tag="pp")
                    for din in range(8):
                        nc.tensor.matmul(
                            ps[:], wk[dout // 4][:, dout % 4, din, :],
                            it[:, din, :], start=(din == 0), stop=(din == 7))
                    # k1 = d*c + psum, with a single fp32r rounding
                    nc.vector.scalar_tensor_tensor(
                        k1T[:, dout, sb * 512:(sb + 1) * 512],
                        dkb[:, sb * 512:(sb + 1) * 512],
                        ck[:, dout:dout + 1], ps[:],
                        op0=ALU.mult, op1=ALU.add)
                    if sb == 0 and dout == 0:
                        first_evac()
                    if sb == 1 and dout == 5:
                        wv[0] = load_whalf(wv0_p, Wv, 0, "wv0")
            dkp.release()
            wk1_p.release()
            wk0_p.release()

            # =============== phase V: v1 = vT^T Wv (no correction) ===============
            wv1_p = tc.alloc_tile_pool(name="wv1", bufs=1, side="right")
            wv[1] = load_whalf(wv1_p, Wv, 1, "wv1")
            wq = [None, None]
            wq_pool = tc.alloc_tile_pool(name="wq", bufs=1, side="left")
            for sb in range(4):
                it = load_in(vT, sb)
                for kc in range(4):
                    ps0 = pps.tile([128, 512], f32, tag="pp")
                    ps1 = pps.tile([128, 512], f32, tag="pp")
                    for din in range(8):
                        lhs = it[:, din, kc * 128:(kc + 1) * 128]
                        nc.tensor.matmul(ps0[:], lhs, wv[0][:, :, din, :],
                                         start=(din == 0), stop=(din == 7))
                        nc.tensor.matmul(ps1[:], lhs, wv[1][:, :, din, :],
                                         start=(din == 0), stop=(din == 7))
                    # evacuate on Scalar (bf16 cast) to keep Vector free
                    nc.scalar.activation(v1[:, sb * 4 + kc, 0:512], ps0[:], AF.Copy,
                                         bias=0.0, scale=1.0)
                    nc.scalar.activation(v1[:, sb * 4 + kc, 512:1024], ps1[:], AF.Copy,
                                         bias=0.0, scale=1.0)
                if sb == 1:
                    wq[0] = load_whalf(wq_pool, Wq, 0, "wq")
            wv1_p.release()
            wv0_p.release()

            # ====== phase Q: q1T = Wq^T qT (+ fused correction; 1/32 folded into exp) ======
            q1_pool = tc.alloc_tile_pool(name="q1p", bufs=1, side="right")
            q1T = q1_pool.tile([128, 8, SH], f32r, tag="q1T")  # 32 KB/part
            # own pool: the half-2 load must not wait on half-1's last reader
            wq1_p = tc.alloc_tile_pool(name="wq1", bufs=1, side="left")
            wq[1] = load_whalf(wq1_p, Wq, 1, "wq1")
            its = [None, None]
            for wh in range(2):
                w = wq[wh]
                for sb in range(2):
                    if wh == 0:
                        its[sb] = load_in(qT, sb)
                    for d4 in range(4):
                        dout = wh * 4 + d4
                        ps = pps.tile([128, 512], f32, tag="pp")
                        for din in range(8):
                            nc.tensor.matmul(
                                ps[:], w[:, d4, din, :],
                                its[sb][:, din, :], start=(din == 0), stop=(din == 7))
                        nc.vector.scalar_tensor_tensor(
                            q1T[:, dout, sb * 512:(sb + 1) * 512],
                            dqb[:, sb * 512:(sb + 1) * 512],
                            cq[:, dout:dout + 1], ps[:],
                            op0=ALU.mult, op1=ALU.add)
            wq1_p.release()
            wq_pool.release()
            corrK.release()
            inp.release()
            pps.release()

            # ---- attention, one 128-row query tile per slot ----
            with (
                tc.tile_pool(name="work", bufs=2) as work,
                tc.tile_pool(name="small", bufs=2) as small,
                tc.tile_pool(name="tpps", bufs=2, space="PSUM") as tpps,
                tc.tile_pool(name="svps", bufs=2, space="PSUM") as svps,
            ):
                ident = work.tile([128, 128], bf16, tag="ident")
                make_identity(nc, ident[:])
                # small slots mid-stream: their serial softmax latency hides
                # under the bigger slots' matmuls instead of trailing the kernel
                for s in [0, 1, 2, 6, 7, 3, 4, 5]:
                    E = EXT[s]                # extent in 128-chunks
                    L = E * 128               # extent in keys
                    nb5, nb2 = NB512[s], NB256[s]
                    mu = work.tile([128, 2048], u8, tag="mu")
                    nc.gpsimd.dma_start(out=mu[:, :L], in_=Mu[s * 128:(s + 1) * 128, :L])
                    logits = work.tile([128, 2048], f32, tag="lg")
                    for b in range(nb5 + nb2):
                        n = 512 if b < nb5 else 256
                        qk = qkps.tile([128, 512], f32, tag="qk")
                        for din in range(8):
                            nc.tensor.matmul(
                                qk[:, :n],
                                q1T[:, din, s * 128:(s + 1) * 128],
                                k1T[:, din, b * 512:b * 512 + n],
                                start=(din == 0), stop=(din == 7))
                        # logits = mask*2^24 + qk  (allowed ~2^24, masked small)
                        nc.vector.scalar_tensor_tensor(
                            logits[:, b * 512:b * 512 + n], mu[:, b * 512:b * 512 + n],
                            MOFF, qk[:, :n], op0=ALU.mult, op1=ALU.add)
                    negmax = small.tile([128, 1], f32, tag="negmax")
                    nc.vector.tensor_reduce(
                        negmax[:], logits[:, :L], axis=mybir.AxisListType.X,
                        op=ALU.max, negate=True)
                    negmax_s = small.tile([128, 1], f32, tag="negmax_s")
                    nc.vector.tensor_scalar_mul(negmax_s[:], negmax[:], SCALE)
                    # exp((logits - max)/32) + per-block row sums
                    probs = work.tile([128, 16, 128], bf16, tag="probs")
                    p2 = probs[:].rearrange("p a b -> p (a b)")
                    sums = small.tile([128, 4], f32, tag="sums")
                    for b in range(nb5 + nb2):
                        n = 512 if b < nb5 else 256
                        nc.scalar.activation(
                            p2[:, b * 512:b * 512 + n], logits[:, b * 512:b * 512 + n],
                            AF.Exp, bias=negmax_s[:, 0:1], scale=SCALE,
                            accum_out=sums[:, b:b + 1])
                    total = small.tile([128, 1], f32, tag="total")
                    nc.vector.tensor_reduce(
                        total[:], sums[:, :nb5 + nb2], axis=mybir.AxisListType.X,
                        op=ALU.add)
                    recip = small.tile([128, 1], f32, tag="recip")
                    nc.vector.reciprocal(recip[:], total[:])
                    # transpose probs 128x128 blocks (PE), evacuate on Scalar
                    pT = work.tile([128, 16, 128], bf16, tag="pT")
                    for j in range(E):
                        tp = tpps.tile([128, 128], bf16, tag="tp")
                        nc.tensor.transpose(tp[:], probs[:, j, :], ident[:])
                        nc.scalar.activation(pT[:, j, :], tp[:], AF.Copy,
                                             bias=0.0, scale=1.0)
                    # SV: out[q, dv] = sum_j pT[j].T @ v1[j, dv]
                    ot = work.tile([128, D], f32, tag="ot")
                    sv0 = svps.tile([128, 512], f32, tag="sv")
                    sv1 = svps.tile([128, 512], f32, tag="sv")
                    for j in range(E):
                        nc.tensor.matmul(sv0[:], pT[:, j, :], v1[:, j, 0:512],
                                         start=(j == 0), stop=(j == E - 1))
                        nc.tensor.matmul(sv1[:], pT[:, j, :], v1[:, j, 512:1024],
                                         start=(j == 0), stop=(j == E - 1))
                    # normalize by 1/rowsum during evacuation
                    nc.scalar.activation(ot[:, 0:512], sv0[:], AF.Copy,
                                         bias=0.0, scale=recip[:, 0:1])
                    nc.scalar.activation(ot[:, 512:1024], sv1[:], AF.Copy,
                                         bias=0.0, scale=recip[:, 0:1])
                    nc.sync.dma_start(out=O[s * 128:(s + 1) * 128, :], in_=ot[:])
            q1_pool.release()
            qkps.release()
    nc.finalize()
    return nc


_NC_CACHE = []


def kernel(q, k, v, mask, W_q, W_k, W_v):
    q = np.asarray(q, dtype=np.float32)
    k = np.asarray(k, dtype=np.float32)
    v = np.asarray(v, dtype=np.float32)
    W_q = np.asarray(W_q, dtype=np.float32)
    W_k = np.asarray(W_k, dtype=np.float32)
    W_v = np.asarray(W_v, dtype=np.float32)
    mask_u8 = np.asarray(mask).astype(np.uint8)

    if not _NC_CACHE:
        _NC_CACHE.append(build())
    nc = _NC_CACHE[0]

    # host-side rank-1 f32r correction factors
    # d[s] = sum_din (x[din, s] - rne11(x)[din, s]); c[dout] = colmean(rne11(W))
    ckm = rne11(W_k).mean(axis=0, dtype=np.float64).astype(np.float32)  # [D]
    cqm = rne11(W_q).mean(axis=0, dtype=np.float64).astype(np.float32)
    CKa = np.ascontiguousarray(ckm.reshape(8, 128).T)                   # [128, 8]
    CQa = np.ascontiguousarray(cqm.reshape(8, 128).T)
    WkH = relay_w(W_k)
    WqH = relay_w(W_q)
    WvH = relay_w(W_v)
    kH, vH, dkH = {}, {}, {}
    for b in range(B):
        kH[b] = relay(k[b].T, 4)
        vH[b] = relay(v[b].T, 4)
        dkH[b] = (k[b] - rne11(k[b])).sum(axis=1, dtype=np.float64).astype(np.float32)

    row_sets = []
    in_maps = []
    for c in range(8):
        b, cls = c // 2, c % 2
        rows = np.concatenate([np.arange(128 * t, 128 * (t + 1)) for t in TILES[cls]])
        row_sets.append((b, rows))
        qb = q[b][rows, :]
        dq = (qb - rne11(qb)).sum(axis=1, dtype=np.float64).astype(np.float32)  # [SH]
        in_maps.append({
            "qT": relay(qb.T, 2),
            "kT": kH[b],
            "vT": vH[b],
            "Wq": WqH, "Wk": WkH, "Wv": WvH,
            "Mu": np.ascontiguousarray(mask_u8[b][rows, :]),
            "DK": np.ascontiguousarray(np.broadcast_to(dkH[b], (128, S))),
            "DQ": np.ascontiguousarray(np.broadcast_to(dq, (128, SH))),
            "CK": CKa, "CQ": CQa,
        })

    res = run_bass_kernel_spmd(nc, in_maps, core_ids=list(range(8)))

    out = np.empty((B, S, D), dtype=np.float32)
    for c in range(8):
        b, rows = row_sets[c]
        out[b][rows, :] = res.results[c]["O"]
    return out


# revision 15
# speedup vs baseline: 1.0027x; 1.0027x over previous
"""Causal dot-product attention (B=4, S=2048, D=1024) on 8 TRN2 NeuronCores.

v2: host-side rank-1 f32r-rounding correction, balanced causal tiling,
per-partition-contiguous input layout (128-descriptor DMAs), host
pre-rounding to f32r (no cast-DMAs -> any engine can issue loads),
evacuations split across Scalar/Vector engines.

Sharding: batch x query-tile-class. Core c handles batch c//2; the 16
query row-tiles (128 rows each) are split so slot s of class 0 gets tile
15-2s (extent 16-2s chunks) and class 1 gets tile 14-2s (extent 15-2s,
padded to 16-2s) -> one SPMD program, near-balanced work.

Numerics: projections and QK^T run in f32r (11-bit-mantissa RNE, full PE
speed); inputs/weights are pre-rounded to the identical RNE-11 grid on
the host. The dominant rounding error of the K/Q projections is the
rank-1 term  rowsum(x - rne11(x)) (x) colmean(rne11(W)); both factors
are computed on the host and shipped as small inputs DK/DQ/CK/CQ. The
evacuation fuses  out = d*c + psum  into one scalar_tensor_tensor.
1/sqrt(D) is applied inside the exp activation; the causal mask comes in
as a fused (mask*2^24 + logits) op before max-subtraction.
"""
import numpy as np
import concourse.bass as bass
import concourse.mybir as mybir
from concourse import bacc
from concourse.tile import TileContext
from concourse.bass_utils import run_bass_kernel_spmd
from concourse.masks import make_identity

f32 = mybir.dt.float32
f32r = mybir.dt.float32r
bf16 = mybir.dt.bfloat16
u8 = mybir.dt.uint8
AF = mybir.ActivationFunctionType
ALU = mybir.AluOpType

B, S, D = 4, 2048, 1024
SH = 1024                  # query rows per core
NSLOT = 8
EXT = [16, 14, 12, 10, 8, 6, 4, 2]        # key extent per slot, 128-chunks
NB512 = [e // 4 for e in EXT]             # full 512 blocks
NB256 = [(e % 4) // 2 for e in EXT]       # trailing 256 block (0 or 1)
TILES = [[15, 13, 11, 9, 7, 5, 3, 1], [14, 12, 10, 8, 6, 4, 2, 0]]
MOFF = float(2 ** 24)
SCALE = 1.0 / 32.0


def rne11(x):
    """Bit-exact f32r rounding: RNE to 11 mantissa bits."""
    b = np.asarray(x, dtype=np.float32).view(np.uint32).astype(np.uint64)
    half = np.uint64(1 << 11)
    lsb = (b >> np.uint64(12)) & np.uint64(1)
    b2 = ((b + half - np.uint64(1) + lsb) >> np.uint64(12)) << np.uint64(12)
    return b2.astype(np.uint32).view(np.float32)


def relay(xT, nsb):
    """[D, N] -> [nsb, 128, 8, N//nsb] per-partition-contiguous, rne11'd."""
    Dn, N = xT.shape
    w = N // nsb
    out = rne11(xT).reshape(8, 128, nsb, w).transpose(2, 1, 0, 3)
    return np.ascontiguousarray(out)


def relay_w(W):
    """[D, D] -> [2, 4, 128, 8, 128] chunked per dout-group, rne11'd."""
    out = rne11(W).reshape(8, 128, 2, 4, 128).transpose(2, 3, 1, 0, 4)
    return np.ascontiguousarray(out)


def build():
    nc = bacc.Bacc()
    qT = nc.dram_tensor("qT", [2, 128, 8, 512], f32r, kind="ExternalInput")
    kT = nc.dram_tensor("kT", [4, 128, 8, 512], f32r, kind="ExternalInput")
    vT = nc.dram_tensor("vT", [4, 128, 8, 512], f32r, kind="ExternalInput")
    Wq = nc.dram_tensor("Wq", [2, 4, 128, 8, 128], f32r, kind="ExternalInput")
    Wk = nc.dram_tensor("Wk", [2, 4, 128, 8, 128], f32r, kind="ExternalInput")
    Wv = nc.dram_tensor("Wv", [2, 4, 128, 8, 128], f32r, kind="ExternalInput")
    Mu = nc.dram_tensor("Mu", [SH, S], u8, kind="ExternalInput")
    DK = nc.dram_tensor("DK", [128, S], f32, kind="ExternalInput")
    DQ = nc.dram_tensor("DQ", [128, SH], f32, kind="ExternalInput")
    CK = nc.dram_tensor("CK", [128, 8], f32, kind="ExternalInput")
    CQ = nc.dram_tensor("CQ", [128, 8], f32, kind="ExternalInput")
    O = nc.dram_tensor("O", [SH, D], f32, kind="ExternalOutput")

    def load_whalf(pool, W5, half, tag, eng=None):
        w = pool.tile([128, 4, 8, 128], f32r, tag=tag)
        for d4 in range(4):
            (eng or nc.gpsimd).dma_start(out=w[:, d4], in_=W5[half, d4])
        return w

    with TileContext(nc) as tc:
        with tc.tile_pool(name="pers", bufs=1) as pers:
            k1T = pers.tile([128, 8, S], f32r, tag="k1T")      # 64 KB/part
            v1 = pers.tile([128, 16, D], bf16, tag="v1")       # 32 KB/part

            inp = tc.alloc_tile_pool(name="inp", bufs=2, side="left")
            corrK = tc.alloc_tile_pool(name="corr", bufs=1, side="left")
            wk0_p = tc.alloc_tile_pool(name="wk0", bufs=1, side="left")
            wk1_p = tc.alloc_tile_pool(name="wk1", bufs=1, side="left")
            dkp = tc.alloc_tile_pool(name="dkp", bufs=1, side="left")
            wv0_p = tc.alloc_tile_pool(name="wv0", bufs=1, side="right")

            pps = tc.alloc_tile_pool(name="pps", bufs=8, space="PSUM")

            def load_in(X4, sb, eng=None):
                """One [128, 8, 512] f32r input chunk, single 128-desc DMA."""
                it = inp.tile([128, 8, 512], f32r, tag="inT")
                (eng or nc.sync).dma_start(out=it, in_=X4[sb])
                return it

            # host-computed rank-1 correction factors
            dkb = dkp.tile([128, S], f32, tag="dkb")       # 8 KB/part
            dqb = corrK.tile([128, SH], f32, tag="dqb")     # 4 KB/part
            ck = corrK.tile([128, 8], f32, tag="ck")
            cq = corrK.tile([128, 8], f32, tag="cq")
            # =============== phase K: k1T = Wk^T kT (+ fused correction) ===============
            # startup-gating set spread over the three DMA-capable queues;
            # correction factors queue AFTER it (needed only at first evac)
            wk0 = wk0_p.tile([128, 4, 8, 128], f32r, tag="wk0")
            nc.gpsimd.dma_start(out=wk0[:, 0], in_=Wk[0, 0])
            it0 = inp.tile([128, 8, 512], f32r, tag="inT")
            nc.sync.dma_start(out=wk0[:, 2], in_=Wk[0, 2])
            nc.scalar.dma_start(out=wk0[:, 3], in_=Wk[0, 3])
            nc.gpsimd.dma_start(out=wk0[:, 1], in_=Wk[0, 1])
            nc.sync.dma_start(out=it0[:, 0:4, :], in_=kT[0, :, 0:4])
            nc.scalar.dma_start(out=it0[:, 4:8, :], in_=kT[0, :, 4:8])
            nc.scalar.dma_start(out=ck[:], in_=CK[:, :])
            nc.scalar.dma_start(out=dkb[:, 0:512], in_=DK[:, 0:512])
            wk = [wk0, None]
            wv = [None, None]
            its_k = [it0, None, None, None]

            def first_evac():
                """Everything queued after the gates waits for the first evac,
                so startup-gating transfers (wk0+it0) get the full bandwidth."""
                for eng, tg in ((nc.sync, "gate_s"), (nc.gpsimd, "gate_g"),
                                (nc.scalar, "gate_a")):
                    g = corrK.tile([1, 16], f32r, tag=tg)
                    eng.dma_start(out=g[:], in_=k1T[0:1, 0, 0:16])
                # gated prefetch: inputs on sync, weights+mask on gpsimd
                nc.scalar.dma_start(out=cq[:], in_=CQ[:, :])
                nc.scalar.dma_start(out=dkb[:, 512:], in_=DK[:, 512:])
                nc.scalar.dma_start(out=dqb[:], in_=DQ[:, :])
                wk[1] = load_whalf(wk1_p, Wk, 1, "wk1")
                for sb2 in range(1, 4):
                    its_k[sb2] = load_in(kT, sb2)

            for sb in range(4):
                it = its_k[sb]
                for dout in range(8):
                    ps = pps.tile([128, 512], f32, tag="pp")
                    for din in range(8):
                        nc.tensor.matmul(
                            ps[:], wk[dout // 4][:, dout % 4, din, :],
                            it[:, din, :], start=(din == 0), stop=(din == 7))
                    # k1 = d*c + psum, with a single fp32r rounding
                    nc.vector.scalar_tensor_tensor(
                        k1T[:, dout, sb * 512:(sb + 1) * 512],
                        dkb[:, sb * 512:(sb + 1) * 512],
                        ck[:, dout:dout + 1], ps[:],
                        op0=ALU.mult, op1=ALU.add)
                    if sb == 0 and dout == 0:
                        first_evac()
                    if sb == 1 and dout == 5:
                        wv[0] = load_whalf(wv0_p, Wv, 0, "wv0")
            dkp.release()
            wk1_p.release()
            wk0_p.release()

            # =============== phase V: v1 = vT^T Wv (no correction) ===============
            wv1_p = tc.alloc_tile_pool(name="wv1", bufs=1, side="right")
            wv[1] = load_whalf(wv1_p, Wv, 1, "wv1")
            wq = [None, None]
            wq_pool = tc.alloc_tile_pool(name="wq", bufs=1, side="left")
            for sb in range(4):
                it = load_in(vT, sb)
                for kc in range(4):
                    ps0 = pps.tile([128, 512], f32, tag="pp")
                    ps1 = pps.tile([128, 512], f32, tag="pp")
                    for din in range(8):
                        lhs = it[:, din, kc * 128:(kc + 1) * 128]
                        nc.tensor.matmul(ps0[:], lhs, wv[0][:, :, din, :],
                                         start=(din == 0), stop=(din == 7))
                        nc.tensor.matmul(ps1[:], lhs, wv[1][:, :, din, :],
                                         start=(din == 0), stop=(din == 7))
                    # evacuate on Scalar (bf16 cast) to keep Vector free
                    nc.scalar.activation(v1[:, sb * 4 + kc, 0:512], ps0[:], AF.Copy,
                                         bias=0.0, scale=1.0)
                    nc.scalar.activation(v1[:, sb * 4 + kc, 512:1024], ps1[:], AF.Copy,
                                         bias=0.0, scale=1.0)
                if sb == 1:
                    wq[0] = load_whalf(wq_pool, Wq, 0, "wq")
            wv1_p.release()
            wv0_p.release()

            # ====== phase Q: q1T = Wq^T qT (+ fused correction; 1/32 folded into exp) ======
            q1_pool = tc.alloc_tile_pool(name="q1p", bufs=1, side="right")
            q1T = q1_pool.tile([128, 8, SH], f32r, tag="q1T")  # 32 KB/part
            # own pool: the half-2 load must not wait on half-1's last reader
            wq1_p = tc.alloc_tile_pool(name="wq1", bufs=1, side="left")
            wq[1] = load_whalf(wq1_p, Wq, 1, "wq1")
            its = [None, None]
            for wh in range(2):
                w = wq[wh]
                for sb in range(2):
                    if wh == 0:
                        its[sb] = load_in(qT, sb)
                    for d4 in range(4):
                        dout = wh * 4 + d4
                        ps = pps.tile([128, 512], f32, tag="pp")
                        for din in range(8):
                            nc.tensor.matmul(
                                ps[:], w[:, d4, din, :],
                                its[sb][:, din, :], start=(din == 0), stop=(din == 7))
                        nc.vector.scalar_tensor_tensor(
                            q1T[:, dout, sb * 512:(sb + 1) * 512],
                            dqb[:, sb * 512:(sb + 1) * 512],
                            cq[:, dout:dout + 1], ps[:],
                            op0=ALU.mult, op1=ALU.add)
            wq1_p.release()
            wq_pool.release()
            corrK.release()
            inp.release()
            pps.release()

            # ---- attention, one 128-row query tile per slot ----
            with (
                tc.tile_pool(name="work", bufs=2) as work,
                tc.tile_pool(name="small", bufs=2) as small,
                tc.tile_pool(name="qkps", bufs=3, space="PSUM") as qkps,
                tc.tile_pool(name="tpps", bufs=2, space="PSUM") as tpps,
                tc.tile_pool(name="svps", bufs=2, space="PSUM") as svps,
            ):
                ident = work.tile([128, 128], bf16, tag="ident")
                make_identity(nc, ident[:])
                # small slots mid-stream: their serial softmax latency hides
                # under the bigger slots' matmuls instead of trailing the kernel
                for s in [0, 1, 2, 6, 7, 3, 4, 5]:
                    E = EXT[s]                # extent in 128-chunks
                    L = E * 128               # extent in keys
                    nb5, nb2 = NB512[s], NB256[s]
                    mu = work.tile([128, 2048], u8, tag="mu")
                    nc.gpsimd.dma_start(out=mu[:, :L], in_=Mu[s * 128:(s + 1) * 128, :L])
                    logits = work.tile([128, 2048], f32, tag="lg")
                    for b in range(nb5 + nb2):
                        n = 512 if b < nb5 else 256
                        qk = qkps.tile([128, 512], f32, tag="qk")
                        for din in range(8):
                            nc.tensor.matmul(
                                qk[:, :n],
                                q1T[:, din, s * 128:(s + 1) * 128],
                                k1T[:, din, b * 512:b * 512 + n],
                                start=(din == 0), stop=(din == 7))
                        # logits = mask*2^24 + qk  (allowed ~2^24, masked small)
                        nc.vector.scalar_tensor_tensor(
                            logits[:, b * 512:b * 512 + n], mu[:, b * 512:b * 512 + n],
                            MOFF, qk[:, :n], op0=ALU.mult, op1=ALU.add)
                    negmax = small.tile([128, 1], f32, tag="negmax")
                    nc.vector.tensor_reduce(
                        negmax[:], logits[:, :L], axis=mybir.AxisListType.X,
                        op=ALU.max, negate=True)
                    negmax_s = small.tile([128, 1], f32, tag="negmax_s")
                    nc.vector.tensor_scalar_mul(negmax_s[:], negmax[:], SCALE)
                    # exp((logits - max)/32) + per-block row sums
                    probs = work.tile([128, 16, 128], bf16, tag="probs")
                    p2 = probs[:].rearrange("p a b -> p (a b)")
                    sums = small.tile([128, 4], f32, tag="sums")
                    for b in range(nb5 + nb2):
                        n = 512 if b < nb5 else 256
                        nc.scalar.activation(
                            p2[:, b * 512:b * 512 + n], logits[:, b * 512:b * 512 + n],
                            AF.Exp, bias=negmax_s[:, 0:1], scale=SCALE,
                            accum_out=sums[:, b:b + 1])
                    total = small.tile([128, 1], f32, tag="total")
                    nc.vector.tensor_reduce(
                        total[:], sums[:, :nb5 + nb2], axis=mybir.AxisListType.X,
                        op=ALU.add)
                    recip = small.tile([128, 1], f32, tag="recip")
                    nc.vector.reciprocal(recip[:], total[:])
                    # transpose probs 128x128 blocks (PE), evacuate on Scalar
                    pT = work.tile([128, 16, 128], bf16, tag="pT")
                    for j in range(E):
                        tp = tpps.tile([128, 128], bf16, tag="tp")
                        nc.tensor.transpose(tp[:], probs[:, j, :], ident[:])
                        nc.scalar.activation(pT[:, j, :], tp[:], AF.Copy,
                                             bias=0.0, scale=1.0)
                    # SV: out[q, dv] = sum_j pT[j].T @ v1[j, dv]
                    ot = work.tile([128, D], f32, tag="ot")
                    sv0 = svps.tile([128, 512], f32, tag="sv")
                    sv1 = svps.tile([128, 512], f32, tag="sv")
                    for j in range(E):
                        nc.tensor.matmul(sv0[:], pT[:, j, :], v1[:, j, 0:512],
                                         start=(j == 0), stop=(j == E - 1))
                        nc.tensor.matmul(sv1[:], pT[:, j, :], v1[:, j, 512:1024],
                                         start=(j == 0), stop=(j == E - 1))
                    # normalize by 1/rowsum during evacuation
                    nc.scalar.activation(ot[:, 0:512], sv0[:], AF.Copy,
                                         bias=0.0, scale=recip[:, 0:1])
                    nc.scalar.activation(ot[:, 512:1024], sv1[:], AF.Copy,
                                         bias=0.0, scale=recip[:, 0:1])
                    nc.sync.dma_start(out=O[s * 128:(s + 1) * 128, :], in_=ot[:])
            q1_pool.release()
    nc.finalize()
    return nc


_NC_CACHE = []


def kernel(q, k, v, mask, W_q, W_k, W_v):
    q = np.asarray(q, dtype=np.float32)
    k = np.asarray(k, dtype=np.float32)
    v = np.asarray(v, dtype=np.float32)
    W_q = np.asarray(W_q, dtype=np.float32)
    W_k = np.asarray(W_k, dtype=np.float32)
    W_v = np.asarray(W_v, dtype=np.float32)
    mask_u8 = np.asarray(mask).astype(np.uint8)

    if not _NC_CACHE:
        _NC_CACHE.append(build())
    nc = _NC_CACHE[0]

    # host-side rank-1 f32r correction factors
    # d[s] = sum_din (x[din, s] - rne11(x)[din, s]); c[dout] = colmean(rne11(W))
    ckm = rne11(W_k).mean(axis=0, dtype=np.float64).astype(np.float32)  # [D]
    cqm = rne11(W_q).mean(axis=0, dtype=np.float64).astype(np.float32)
    CKa = np.ascontiguousarray(ckm.reshape(8, 128).T)                   # [128, 8]
    CQa = np.ascontiguousarray(cqm.reshape(8, 128).T)
    WkH = relay_w(W_k)
    WqH = relay_w(W_q)
    WvH = relay_w(W_v)
    kH, vH, dkH = {}, {}, {}
    for b in range(B):
        kH[b] = relay(k[b].T, 4)
        vH[b] = relay(v[b].T, 4)
        dkH[b] = (k[b] - rne11(k[b])).sum(axis=1, dtype=np.float64).astype(np.float32)

    row_sets = []
    in_maps = []
    for c in range(8):
        b, cls = c // 2, c % 2
        rows = np.concatenate([np.arange(128 * t, 128 * (t + 1)) for t in TILES[cls]])
        row_sets.append((b, rows))
        qb = q[b][rows, :]
        dq = (qb - rne11(qb)).sum(axis=1, dtype=np.float64).astype(np.float32)  # [SH]
        in_maps.append({
            "qT": relay(qb.T, 2),
            "kT": kH[b],
            "vT": vH[b],
            "Wq": WqH, "Wk": WkH, "Wv": WvH,
            "Mu": np.ascontiguousarray(mask_u8[b][rows, :]),
            "DK": np.ascontiguousarray(np.broadcast_to(dkH[b], (128, S))),
            "DQ": np.ascontiguousarray(np.broadcast_to(dq, (128, SH))),
            "CK": CKa, "CQ": CQa,
        })

    res = run_bass_kernel_spmd(nc, in_maps, core_ids=list(range(8)))

    out = np.empty((B, S, D), dtype=np.float32)
    for c in range(8):
        b, rows = row_sets[c]
        out[b][rows, :] = res.results[c]["O"]
    return out
